# revision 1
# baseline (speedup 1.0000x reference)
"""Trainium2 Bass kernel for ConvMultiHeadAttention (N=16, L=1024, E=512, H=8).

Data-parallel over batch: 8 NeuronCores x 2 batches each. Per core:
transposed-layout projections (fp16/fp32 mix), S^T = K_h^T-contract-d Q_h
attention logits with softmax-over-partitions via an appended ones column
in the AV matmul (denominator comes out as row 64 of the O accumulator),
reciprocals via exp(-ln d) on ScalarE, selector-matmul partition broadcast,
and a final fused output projection + bias.
"""

import numpy as np
import concourse.bass as bass
import concourse.mybir as mybir
import concourse.tile as tile
from contextlib import ExitStack
from concourse import bacc

P = 128
L = 1024
E = 512
H = 8
D = 64
NB = 2            # batches per core
TT = L // P       # 8 token tiles per batch
EPO = E // P      # 4 e-subtiles
FP32 = mybir.dt.float32
FP32R = mybir.dt.float16  # fp16 variant
BF16 = mybir.dt.float16
AF = mybir.ActivationFunctionType
ALU = mybir.AluOpType


def host_constants():
    import ml_dtypes
    ident = np.eye(P, dtype=np.float16)
    # sel2[p, 64h + j] = 1 iff p == 32*(h % 4): picks denom row of head h
    sel2 = np.zeros((P, H * D), np.float32)
    for h in range(H):
        sel2[32 * (h % 4), h * D:(h + 1) * D] = 1.0
    return ident, sel2


def build(debug=False):
    nc = bacc.Bacc("TRN2", target_bir_lowering=False, debug=debug)
    q_d = nc.dram_tensor("q", [NB, L, E], FP32, kind="ExternalInput").ap()
    k_d = nc.dram_tensor("k", [NB, L, E], FP32, kind="ExternalInput").ap()
    v_d = nc.dram_tensor("v", [NB, L, E], FP32, kind="ExternalInput").ap()
    wq_d = nc.dram_tensor("Wq", [E, E], FP32, kind="ExternalInput").ap()
    wk_d = nc.dram_tensor("Wk", [E, E], FP32, kind="ExternalInput").ap()
    wv_d = nc.dram_tensor("Wv", [E, E], FP32, kind="ExternalInput").ap()
    wo_d = nc.dram_tensor("Wo", [E, E], FP32, kind="ExternalInput").ap()
    bo_d = nc.dram_tensor("bo_bcast", [P, E], FP32, kind="ExternalInput").ap()
    id_d = nc.dram_tensor("ident", [P, P], BF16, kind="ExternalInput").ap()
    sel_d = nc.dram_tensor("sel2", [P, H * D], FP32, kind="ExternalInput").ap()
    out_d = nc.dram_tensor("out", [NB, L, E], FP32, kind="ExternalOutput").ap()

    with tile.TileContext(nc) as tc, ExitStack() as ctx:
        consts = ctx.enter_context(tc.tile_pool(name="consts", bufs=1))
        wt_pool = ctx.enter_context(tc.tile_pool(name="wt", bufs=1))
        xin_pool = ctx.enter_context(tc.tile_pool(name="xin", bufs=4))
        xt_pool = ctx.enter_context(tc.tile_pool(name="xt", bufs=3))
        qk_pool = ctx.enter_context(tc.tile_pool(name="qk", bufs=2))
        vh_pool = ctx.enter_context(tc.tile_pool(name="vh", bufs=2))
        st_pool = ctx.enter_context(tc.tile_pool(name="st", bufs=1))
        p_pool = ctx.enter_context(tc.tile_pool(name="pp", bufs=12))
        dn_pool = ctx.enter_context(tc.tile_pool(name="dn", bufs=1))
        o_pool = ctx.enter_context(tc.tile_pool(name="oo", bufs=3))
        ps_mm = ctx.enter_context(tc.tile_pool(name="psmm", bufs=2, space="PSUM"))
        ps_s = ctx.enter_context(tc.tile_pool(name="pss", bufs=2, space="PSUM"))
        ps_o = ctx.enter_context(tc.tile_pool(name="pso", bufs=2, space="PSUM"))

        # ---- constants ----
        ident = consts.tile([P, P], BF16)
        nc.sync.dma_start(ident[:], id_d)
        sel_f = xin_pool.tile([P, H * D], FP32, tag="xin")
        nc.sync.dma_start(sel_f[:], sel_d)
        sel = consts.tile([P, H * D], FP32R)
        nc.vector.tensor_copy(sel[:], sel_f[:])
        bo_t = consts.tile([P, E], FP32)
        nc.sync.dma_start(bo_t[:], bo_d)

        # ---- weight transposes: W [f, e] -> WT [e(pi), epo, f] fp32r ----
        wts = {}
        for wname, w_d in [("q", wq_d), ("k", wk_d), ("v", wv_d), ("o", wo_d)]:
            w_raw = xt_pool.tile([P, EPO, E], FP32, tag="xt")
            nc.sync.dma_start(w_raw[:], w_d.rearrange("(fo fi) e -> fi fo e", fi=P))
            w_nat = xt_pool.tile([P, EPO, E], BF16, tag="xtb")
            nc.vector.tensor_copy(w_nat[:], w_raw[:])
            wt = wt_pool.tile([P, EPO, E], FP32R, tag=f"wt_{wname}")
            for epo in range(EPO):
                ps = ps_mm.tile([P, E], BF16, tag="mm")
                for fpo in range(EPO):
                    nc.tensor.transpose(
                        ps[:, fpo * P:(fpo + 1) * P],
                        w_nat[:, fpo, epo * P:(epo + 1) * P],
                        ident[:],
                    )
                if wname == "q":
                    # fold 1/sqrt(D) into Wq
                    nc.vector.tensor_scalar_mul(wt[:, epo, :], ps[:], 1.0 / np.sqrt(D))
                else:
                    nc.vector.tensor_copy(wt[:, epo, :], ps[:])
            wts[wname] = wt

        out_tiles = []
        preps = {}
        for b in range(NB):
            # ======== prep: transposes + projections ========
            xts = {}
            for tname, x_d in [("q", q_d), ("k", k_d), ("v", v_d)]:
                xt = xt_pool.tile([P, EPO, L], FP32R, tag="xt")
                for tt in range(TT):
                    xin = xin_pool.tile([P, E], FP32, tag="xin")
                    nc.sync.dma_start(xin[:], x_d[b, tt * P:(tt + 1) * P, :])
                    xin_b = xin_pool.tile([P, E], BF16, tag="xinb")
                    nc.vector.tensor_copy(xin_b[:], xin[:])
                    ps = ps_mm.tile([P, E], BF16, tag="mm")
                    for epo in range(EPO):
                        nc.tensor.transpose(
                            ps[:, epo * P:(epo + 1) * P],
                            xin_b[:, epo * P:(epo + 1) * P],
                            ident[:],
                        )
                    # ps is [e-chunk x 4, t] blocks: block epo holds x^T[e(epo), t-tile]
                    nc.scalar.copy(
                        xt[:, :, tt * P:(tt + 1) * P],
                        ps[:].rearrange("p (epo t) -> p epo t", epo=EPO),
                    )
                xts[tname] = xt

            # qh^T, kh^T: [f(pi), fpo, t] = WT_x^T-contract-e @ x^T
            qkts = {}
            for tname in ["q", "k"]:
                wt = wts[tname]
                xt = xts[tname]
                ht = qk_pool.tile([P, EPO, L], FP32R, tag=f"ht_{tname}")
                for fpo in range(EPO):
                    for tch in range(L // E):  # 2 chunks of 512
                        ps = ps_mm.tile([P, E], FP32, tag="mm")
                        for epo in range(EPO):
                            nc.tensor.matmul(
                                ps[:],
                                wt[:, epo, fpo * P:(fpo + 1) * P],
                                xt[:, epo, tch * E:(tch + 1) * E],
                                start=(epo == 0),
                                stop=(epo == EPO - 1),
                            )
                        nc.vector.tensor_copy(ht[:, fpo, tch * E:(tch + 1) * E], ps[:])
                qkts[tname] = ht

            # vh natural [t(pi), tt, h, 65]; col 64 = ones
            vh = vh_pool.tile([P, TT, H, D + 1], FP32R, tag="vh")
            nc.vector.memset(vh[:], 1.0)  # ones col at [:,:,:,D]; rest overwritten
            wt = wts["v"]
            xt = xts["v"]
            for tt in range(TT):
                ps = ps_mm.tile([P, E], FP32, tag="mm")
                for epo in range(EPO):
                    nc.tensor.matmul(
                        ps[:],
                        xt[:, epo, tt * P:(tt + 1) * P],
                        wt[:, epo, :],
                        start=(epo == 0),
                        stop=(epo == EPO - 1),
                    )
                nc.vector.tensor_copy(
                    vh[:, tt, :, 0:D],
                    ps[:].rearrange("p (h d) -> p h d", h=H),
                )

            preps[b] = (qkts, vh)

        for b in range(NB):
            # ======== attention ========
            qkts, vh = preps[b]
            qht, kht = qkts["q"], qkts["k"]
            stage = st_pool.tile([P, EPO, L], FP32R, tag="st")
            denom = dn_pool.tile([P, 2, L], FP32, tag="dn")
            nc.vector.memset(denom[:], 1.0)
            def emit_s_exp(h):
                hpo, hoff = h // 2, D * (h % 2)
                pts = []
                for lt in range(TT):
                    pss = ps_s.tile([P, L], FP32, tag="s")
                    for ch in range(L // E):
                        nc.tensor.matmul(
                            pss[:, ch * E:(ch + 1) * E],
                            kht[hoff:hoff + D, hpo, lt * P:(lt + 1) * P],
                            qht[hoff:hoff + D, hpo, ch * E:(ch + 1) * E],
                            start=True,
                            stop=True,
                        )
                    pt = p_pool.tile([P, L], FP32R, tag="p")
                    nc.scalar.activation(pt[:], pss[:], AF.Exp)
                    pts.append(pt)
                return pts

            def emit_av(h, pts):
                hpo, hoff = h // 2, D * (h % 2)
                for ch in range(L // E):
                    pso = ps_o.tile([D + 1, E], FP32, tag="o")
                    for lt in range(TT):
                        nc.tensor.matmul(
                            pso[:],
                            vh[:, lt, h, :],
                            pts[lt][:, ch * E:(ch + 1) * E],
                            start=(lt == 0),
                            stop=(lt == TT - 1),
                        )
                    nc.vector.tensor_copy(
                        stage[hoff:hoff + D, hpo, ch * E:(ch + 1) * E], pso[0:D, :]
                    )
                    nc.vector.tensor_copy(
                        denom[32 * (h % 4):32 * (h % 4) + 1, h // 4, ch * E:(ch + 1) * E],
                        pso[D:D + 1, :],
                    )

            prev = None
            for h in range(H):
                pts = emit_s_exp(h)
                if prev is not None:
                    emit_av(prev[0], prev[1])
                prev = (h, pts)
            emit_av(prev[0], prev[1])

            # recip = exp(-ln(denom)) ; fp32r  (ln computed in place)
            nc.scalar.activation(denom[:], denom[:], AF.Ln)
            recip = dn_pool.tile([P, 2, L], FP32R, tag="dnr")
            nc.scalar.activation(recip[:], denom[:], AF.Exp, scale=-1.0)

            # normalize: stage[head] *= broadcast(recip[h])
            for h in range(H):
                hpo, hoff = h // 2, D * (h % 2)
                psb = ps_s.tile([D, L], FP32, tag="s")
                for ch in range(L // E):
                    nc.tensor.matmul(
                        psb[:, ch * E:(ch + 1) * E],
                        sel[:, h * D:(h + 1) * D],
                        recip[:, h // 4, ch * E:(ch + 1) * E],
                        start=True,
                        stop=True,
                    )
                nc.vector.tensor_tensor(
                    stage[hoff:hoff + D, hpo, :],
                    psb[:],
                    stage[hoff:hoff + D, hpo, :],
                    ALU.mult,
                )

            # ======== output projection ========
            wt = wts["o"]
            for tt in range(TT):
                ps = ps_mm.tile([P, E], FP32, tag="mm")
                for epo in range(EPO):
                    nc.tensor.matmul(
                        ps[:],
                        stage[:, epo, tt * P:(tt + 1) * P],
                        wt[:, epo, :],
                        start=(epo == 0),
                        stop=(epo == EPO - 1),
                    )
                ot = o_pool.tile([P, E], FP32, tag="ot")
                nc.vector.tensor_tensor(ot[:], ps[:], bo_t[:], ALU.add)
                nc.gpsimd.dma_start(out_d[b, tt * P:(tt + 1) * P, :], ot[:])
                out_tiles.append(ot)

    nc.compile()
    return nc




_COMPILED = None


def _get_compiled():
    global _COMPILED
    if _COMPILED is None:
        _COMPILED = build()
    return _COMPILED


def kernel(q, k, v, Wq, Wk, Wv, Wo, bo):
    import numpy as _np

    q = _np.ascontiguousarray(_np.asarray(q, dtype=_np.float32))
    k = _np.ascontiguousarray(_np.asarray(k, dtype=_np.float32))
    v = _np.ascontiguousarray(_np.asarray(v, dtype=_np.float32))
    Wq = _np.ascontiguousarray(_np.asarray(Wq, dtype=_np.float32))
    Wk = _np.ascontiguousarray(_np.asarray(Wk, dtype=_np.float32))
    Wv = _np.ascontiguousarray(_np.asarray(Wv, dtype=_np.float32))
    Wo = _np.ascontiguousarray(_np.asarray(Wo, dtype=_np.float32))
    bo = _np.asarray(bo, dtype=_np.float32)

    nc = _get_compiled()
    ident, sel2 = host_constants()
    bo_bcast = _np.ascontiguousarray(_np.broadcast_to(bo, (P, E)))
    n_cores = 8
    in_maps = []
    for c in range(n_cores):
        in_maps.append({
            "q": _np.ascontiguousarray(q[c * NB:(c + 1) * NB]),
            "k": _np.ascontiguousarray(k[c * NB:(c + 1) * NB]),
            "v": _np.ascontiguousarray(v[c * NB:(c + 1) * NB]),
            "Wq": Wq, "Wk": Wk, "Wv": Wv, "Wo": Wo,
            "bo_bcast": bo_bcast, "ident": ident, "sel2": sel2,
        })

    from concourse.bass_utils import run_bass_kernel_spmd
    res = run_bass_kernel_spmd(nc, in_maps, core_ids=list(range(n_cores)))
    out = _np.concatenate([res.results[c]["out"] for c in range(n_cores)], axis=0)
    return out.astype(_np.float32)

